# revision 54
# baseline (speedup 1.0000x reference)
"""BackgroundForegroundNeRF fused MLP kernel for 8x Trainium2 NeuronCores.

Pure data parallel: the 2M points are split across 8 cores; all weights are
replicated. Per core the network runs feature-major ([feature, point] tiles)
entirely in fp16 (verified 3.6e-4 end-to-end rel err vs the 2e-2 gate):

  x is transposed to feature-major fp16 on the HOST and DMA'd per tile.
  MM1  : W1 (bg_s0 zero-padded | fg_s0 blocks)       -> h1  [128, n]  relu
  MM2  : block-diag(bg_s1, fg_s1)                    -> h2  [128, n]  relu
  C0   : views-part (base-64 zero-padded lhsT, views straight from x)
         + (c0_geo @ s2_geo) @ h2 accumulated in psum (the geo path folds
         into one matrix: no relu between sigma-net output and color-net
         input)                                      -> c0  [128, n]  relu
  C1,C2: block-diag color layers                     relu
  HEADS: per 128-point group, the GROUP ACTIVATIONS become the stationary
         operand and the tiny head weights the moving one:
           pm9s[:, j, 0:3] = h2[:, j*128:+128].T @ w3    (3 moving rows)
           pm9c[:, j, 0:6] = c2[:, j*128:+128].T @ wc3   (6 moving rows)
         so the head outputs materialize POINT-MAJOR directly in psum -
         no feature-major head pass, no PSUM->SBUF copy of a [128, n]
         tile, and no PE transposes. The sigma head runs MID-TILE (it
         only needs h2), so softplus (exp fused into the psum->sbuf
         copy, then ln(1+x)) and the 1/sigma reciprocal all overlap the
         color layers; only the color blend sits after c2.
  Softplus/blend run point-major on 3-9 cols/pt instead of feature-major
  full passes. Relu copies alternate ACT/DVE per instruction; the blend
  muls go to GPSIMD in steady state (DVE for a 1-tile program, where
  Pool's q7 launch latency would sit on the exit path).

All matmuls are single-pass fp16 (1 cyc/row vs fp32's 4; ~3e-4 rel err).
Weights ship as ONE packed [128, 777] fp16 block -> a single DMA (the SP
sequencer pays ~565ns per DMA program, so 9 separate weight DMAs would
cost ~5us of serial prologue). A short chain of dummy matmuls on a
memset tile warms the PE p-state (0.65/1.2 GHz -> 2.4 GHz) while the
weight/input DMAs are still in flight.
"""
import os
import sys

sys.path.insert(0, '/opt/trn_rl_repo')

import numpy as np  # noqa: E402

import concourse.bass as bass  # noqa: E402
import concourse.tile as tile  # noqa: E402
from concourse import mybir  # noqa: E402
from concourse.bass_utils import run_bass_kernel_spmd  # noqa: E402

F32 = mybir.dt.float32
F16 = mybir.dt.float16
AF = mybir.ActivationFunctionType
ALU = mybir.AluOpType

N_CORES = 8
IN_CH, IN_VIEWS, TIME_DIM, HID, GEO = 71, 27, 8, 64, 15
NF = IN_CH + IN_VIEWS            # 98
TILE_PTS = int(os.environ.get('NERF_TILE', '4096'))
CHUNK = int(os.environ.get('NERF_CHUNK', '512'))
MMN = int(os.environ.get('NERF_MMN', '512'))     # psum-bank-sized matmul slices
PSUM_BUFS = int(os.environ.get('NERF_PSUM_BUFS', '7'))
BIGS_BUFS = int(os.environ.get('NERF_BIGS_BUFS', '2'))
IO_BUFS = int(os.environ.get('NERF_IO_BUFS', '3'))
RELU_PAT = os.environ.get('NERF_RELU_PAT', 'VA')  # cycled per relu instr
PM_ENG = os.environ.get('NERF_PM_ENG', 'A')
WARMUP_MMS = int(os.environ.get('NERF_WARMUP', '6'))
WARMUP_COLS = int(os.environ.get('NERF_WARMUP_COLS', '384'))
BLEND_GP = bool(int(os.environ.get('NERF_BLEND_GP', '1')))
XDMA_SPLIT = int(os.environ.get('NERF_XDMA_SPLIT', '2'))

# packed weight block: [128, WB_COLS] fp16, lhsT ([K, M]) layouts
_WOFF = {'w1': 0, 'w2': 128, 'w3': 256, 'wc0e': 259, 'wc0h': 387,
         'wc1': 515, 'wc2': 643, 'wc3': 771}
WB_COLS = 777

LAST_RESULT = None               # BassKernelResults of the last run (for test.py)


def _split_multiwait_instructions(nc, limit=1):
    """The walrus build here rejects instructions with >1 sync wait; hoist
    extra waits onto fresh single-wait NOPs inserted before the instruction."""
    sync_info_cls = None
    for f in nc.m.functions:
        for bb in f.blocks:
            insts = list(bb.instructions)
            if not any(
                i.sync_info is not None and i.sync_info.on_wait
                and len(i.sync_info.on_wait) > limit
                for i in insts
            ):
                continue
            new_list = []
            for inst in insts:
                si = inst.sync_info
                if si is not None and si.on_wait and len(si.on_wait) > limit:
                    if sync_info_cls is None:
                        sync_info_cls = type(si)
                    waits = list(si.on_wait)
                    keep, extra = waits[:limit], waits[limit:]
                    si.on_wait.clear()
                    si.on_wait.extend(keep)
                    for wt in extra:
                        nop = mybir.InstNoOp(
                            name=f"I-mwsplit-{nc.next_id()}", ins=[], outs=[])
                        nop.engine = inst.engine
                        nop.sync_info = sync_info_cls(on_wait=[wt], on_update=[])
                        new_list.append(nop)
                new_list.append(inst)
            while len(bb.instructions):
                bb.instructions.pop()
            for inst in new_list:
                bb.add_instruction(inst)


def _prep_weights(inp):
    """Pack the 14 small MLP weights into one [128, WB_COLS] fp16 block of
    fused lhsT ([K, M]) matrices."""
    g = {k: np.asarray(inp[k], np.float32) for k in inp}
    z = np.zeros

    w1 = z((128, 128), np.float32)              # K=71 -> M=128 (bg|fg h1)
    w1[:63, :64] = g['bg_s0'].T                 # bg uses xyz only (63)
    w1[:71, 64:] = g['fg_s0'].T

    w2 = z((128, 128), np.float32)              # block-diag h1 -> h2
    w2[:64, :64] = g['bg_s1'].T
    w2[64:, 64:] = g['fg_s1'].T

    w3 = z((128, 3), np.float32)                # head logits: bgs, unc, fgs
    w3[:64, 0] = g['bg_s2'][0]
    w3[64:, 1] = g['fg_s2'][1]
    w3[64:, 2] = g['fg_s2'][0]

    # c0 views part, padded so lhsT/rhs sit at base partition 64:
    # rows 64..70 (pts tail in xT) are zero, rows 71..97 are the view dirs.
    wc0e = z((128, 128), np.float32)
    wc0e[71:NF, :64] = g['bg_c0'][:, :IN_VIEWS].T
    wc0e[71:NF, 64:] = g['fg_c0'][:, :IN_VIEWS].T

    # c0 geo part folded through the (linear) sigma-net output: geo enters
    # c0 with no relu in between, so c0_geo @ (s2_geo @ h2) collapses.
    bgp = (g['bg_c0'][:, IN_VIEWS:].astype(np.float64)
           @ g['bg_s2'][1:, :].astype(np.float64)).astype(np.float32)
    fgp = (g['fg_c0'][:, IN_VIEWS:].astype(np.float64)
           @ g['fg_s2'][2:, :].astype(np.float64)).astype(np.float32)
    wc0h = z((128, 128), np.float32)
    wc0h[:64, :64] = bgp.T
    wc0h[64:, 64:] = fgp.T

    wc1 = z((128, 128), np.float32)
    wc1[:64, :64] = g['bg_c1'].T
    wc1[64:, 64:] = g['fg_c1'].T
    wc2 = z((128, 128), np.float32)
    wc2[:64, :64] = g['bg_c2'].T
    wc2[64:, 64:] = g['fg_c2'].T

    wc3 = z((128, 6), np.float32)
    wc3[:64, 0:3] = g['bg_c3'].T
    wc3[64:, 3:6] = g['fg_c3'].T

    wb = z((128, WB_COLS), np.float32)
    for name, mat in [('w1', w1), ('w2', w2), ('w3', w3), ('wc0e', wc0e),
                      ('wc0h', wc0h), ('wc1', wc1), ('wc2', wc2),
                      ('wc3', wc3)]:
        off = _WOFF[name]
        wb[:, off:off + mat.shape[1]] = mat
    return {'wb': wb.astype(np.float16)}


_PROG_CACHE = {}


def _build_program(padded_pts, repeat=None, split_multiwait=True):
    """Build the per-core Bass program for `padded_pts` points."""
    tile_pts = min(TILE_PTS, padded_pts)
    ntiles = padded_pts // tile_pts
    assert ntiles * tile_pts == padded_pts
    ppb = tile_pts // 128
    chunk = min(CHUNK, tile_pts)
    nchunk = tile_pts // chunk
    assert nchunk * chunk == tile_pts
    mmn = min(MMN, chunk)
    nsub = chunk // mmn
    assert nsub * mmn == chunk

    nc = bass.Bass("TRN2", target_bir_lowering=False, debug=False,
                   num_devices=N_CORES)

    xin = nc.dram_tensor("xin", [NF, padded_pts], F16,
                         kind="ExternalInput").ap()
    out = nc.dram_tensor("out", [ntiles * 128, ppb * 6], F32,
                         kind="ExternalOutput").ap()
    wb_dram = nc.dram_tensor("wb", [128, WB_COLS], F16,
                             kind="ExternalInput").ap()

    with tile.TileContext(nc) as tc:
        with tc.tile_pool(name="consts", bufs=1) as consts, \
             tc.tile_pool(name="bigs", bufs=BIGS_BUFS) as bigs, \
             tc.tile_pool(name="io", bufs=IO_BUFS) as io, \
             tc.tile_pool(name="small", bufs=2) as small, \
             tc.tile_pool(name="ps", bufs=PSUM_BUFS, space="PSUM") as ps:

            # p-state warmup first: dummy matmuls on a memset tile so the
            # PE ramps to 2.4 GHz while the input DMAs are still in flight
            # (warmup must not wait on any DMA).
            if WARMUP_MMS:
                wusrc = consts.tile([128, WARMUP_COLS], F16, name="wusrc")
                nc.gpsimd.memset(wusrc, 0.0)
                wu = ps.tile([128, WARMUP_COLS], F32, name="wu", tag="ps")
                for _ in range(WARMUP_MMS):
                    nc.tensor.matmul(wu, wusrc[0:128, 0:128],
                                     wusrc[0:128, 0:WARMUP_COLS],
                                     start=True, stop=True)

            WB = consts.tile([128, WB_COLS], F16, name="sb_wb")
            nc.sync.dma_start(out=WB, in_=wb_dram)
            o = _WOFF
            W1 = WB[0:IN_CH, o['w1']:o['w1'] + 128]
            W2 = WB[0:128, o['w2']:o['w2'] + 128]
            W3 = WB[0:128, o['w3']:o['w3'] + 3]
            WC0E = WB[64:NF, o['wc0e']:o['wc0e'] + 128]
            WC0H = WB[0:128, o['wc0h']:o['wc0h'] + 128]
            WC1 = WB[0:128, o['wc1']:o['wc1'] + 128]
            WC2 = WB[0:128, o['wc2']:o['wc2'] + 128]
            WC3 = WB[0:128, o['wc3']:o['wc3'] + 6]

            relu_ctr = [0]

            def relu_to(dst, src_psum):
                eng = RELU_PAT[relu_ctr[0] % len(RELU_PAT)]
                relu_ctr[0] += 1
                if eng == 'A':
                    nc.scalar.activation(out=dst, in_=src_psum, func=AF.Relu)
                else:
                    nc.vector.tensor_scalar_max(dst, src_psum, 0.0)

            if repeat is None:
                repeat = int(os.environ.get('NERF_REPEAT', '1'))
            for t in [tt for _ in range(repeat) for tt in range(ntiles)]:
                xT = io.tile([NF, tile_pts], F16, name="xT", tag="xT")
                nsp = max(1, min(XDMA_SPLIT, nchunk))
                step = tile_pts // nsp
                cuts = [sp * step for sp in range(nsp)] + [tile_pts]
                for lo_c, hi_c in zip(cuts[:-1], cuts[1:]):
                    nc.sync.dma_start(
                        out=xT[:, lo_c:hi_c],
                        in_=xin[:, t * tile_pts + lo_c:t * tile_pts + hi_c])
                h1r = bigs.tile([128, tile_pts], F16, name="h1r", tag="h1r")
                h2r = bigs.tile([128, tile_pts], F16, name="h2r", tag="h2r")
                c0r = bigs.tile([128, tile_pts], F16, name="c0r", tag="c0r")
                c1r = bigs.tile([128, tile_pts], F16, name="c1r", tag="c1r")
                c2r = bigs.tile([128, tile_pts], F16, name="c2r", tag="c2r")

                gsls = [slice(ch * chunk, (ch + 1) * chunk)
                        for ch in range(nchunk)]

                # stage-major over chunks: each PE wait on a DVE/ACT copy is
                # hidden behind the other chunks' PE work
                p_h1s = []
                for ch in range(nchunk):
                    p_h1 = ps.tile([128, chunk], F32, name="p_h1", tag="ps")
                    for s in range(nsub):
                        msl = slice(s * mmn, (s + 1) * mmn)
                        nc.tensor.matmul(p_h1[:, msl], W1,
                                         xT[0:IN_CH, gsls[ch]][:, msl],
                                         start=True, stop=True)
                    p_h1s.append(p_h1)
                for ch in range(nchunk):
                    relu_to(h1r[:, gsls[ch]], p_h1s[ch])

                p_h2s = []
                for ch in range(nchunk):
                    p_h2 = ps.tile([128, chunk], F32, name="p_h2", tag="ps")
                    for s in range(nsub):
                        msl = slice(s * mmn, (s + 1) * mmn)
                        nc.tensor.matmul(p_h2[:, msl], W2,
                                         h1r[:, gsls[ch]][:, msl],
                                         start=True, stop=True)
                    p_h2s.append(p_h2)
                for ch in range(nchunk):
                    relu_to(h2r[:, gsls[ch]], p_h2s[ch])

                # --- sigma head, mid-tile: it depends only on h2, so the
                # whole softplus/reciprocal chain overlaps the color layers
                # instead of sitting on the exit path. Point-major: the
                # group activations are the matmul STATIONARY operand and
                # the tiny head weights the moving one, so the head output
                # lands point-major in psum (no feature-major pass, no
                # PSUM->SBUF copy of it, no PE transposes).
                pm9s = ps.tile([128, ppb, 3], F32, name="pm9s", tag="ps")
                for j in range(ppb):
                    gsl = slice(j * 128, (j + 1) * 128)
                    nc.tensor.matmul(pm9s[:, j, 0:3], h2r[:, gsl], W3,
                                     start=True, stop=True)
                pmS = small.tile([128, ppb, 3], F32, name="pmS", tag="pmS")
                inv = small.tile([128, ppb], F32, name="inv", tag="inv")
                out_sb = io.tile([128, ppb, 6], F32, name="out_sb",
                                 tag="out_sb")
                # softplus on 3 cols/pt (48 cols, not 4096); exp fuses into
                # the psum->sbuf copy
                nc.scalar.activation(out=pmS, in_=pm9s, func=AF.Exp)
                nc.scalar.activation(out=pmS, in_=pmS, func=AF.Ln, bias=1.0)
                # sigma = bgs + fgs (+1e-9 in the reference; dropped on the
                # Pool path: softplus of an fp32 logit is 0 only if exp
                # underflows at x < -87, and the 1e-9 delta is 12 orders
                # under the absmax gate). The add runs on the otherwise-
                # idle Pool; sigma/unc/fg_sigma cols are final mid-tile.
                if BLEND_GP:
                    nc.gpsimd.tensor_add(out_sb[:, :, 3], pmS[:, :, 0],
                                         pmS[:, :, 2])
                else:
                    nc.vector.scalar_tensor_tensor(
                        out=out_sb[:, :, 3], in0=pmS[:, :, 0], scalar=1e-9,
                        in1=pmS[:, :, 2], op0=ALU.add, op1=ALU.add)
                nc.vector.reciprocal(out=inv, in_=out_sb[:, :, 3])
                # unc/fg_sigma cols: SBUF->SBUF and mid-tile (not on the
                # exit path), so the otherwise-idle Pool takes it
                (nc.gpsimd if BLEND_GP
                 else nc.vector).tensor_copy(out=out_sb[:, :, 4:6],
                                             in_=pmS[:, :, 1:3])

                p_c0s = []
                for ch in range(nchunk):
                    p_c0 = ps.tile([128, chunk], F32, name="p_c0", tag="ps")
                    for s in range(nsub):
                        msl = slice(s * mmn, (s + 1) * mmn)
                        nc.tensor.matmul(p_c0[:, msl], WC0E,
                                         xT[64:NF, gsls[ch]][:, msl],
                                         start=True, stop=False)
                        nc.tensor.matmul(p_c0[:, msl], WC0H,
                                         h2r[:, gsls[ch]][:, msl],
                                         start=False, stop=True)
                    p_c0s.append(p_c0)
                for ch in range(nchunk):
                    relu_to(c0r[:, gsls[ch]], p_c0s[ch])

                p_c1s = []
                for ch in range(nchunk):
                    p_c1 = ps.tile([128, chunk], F32, name="p_c1", tag="ps")
                    for s in range(nsub):
                        msl = slice(s * mmn, (s + 1) * mmn)
                        nc.tensor.matmul(p_c1[:, msl], WC1,
                                         c0r[:, gsls[ch]][:, msl],
                                         start=True, stop=True)
                    p_c1s.append(p_c1)
                for ch in range(nchunk):
                    relu_to(c1r[:, gsls[ch]], p_c1s[ch])

                p_c2s = []
                for ch in range(nchunk):
                    p_c2 = ps.tile([128, chunk], F32, name="p_c2", tag="ps")
                    for s in range(nsub):
                        msl = slice(s * mmn, (s + 1) * mmn)
                        nc.tensor.matmul(p_c2[:, msl], WC2,
                                         c1r[:, gsls[ch]][:, msl],
                                         start=True, stop=True)
                    p_c2s.append(p_c2)
                for ch in range(nchunk):
                    relu_to(c2r[:, gsls[ch]], p_c2s[ch])

                # --- color head (point-major, same stationary trick) ---
                # For a 1-tile program the tail is the exit path: run the
                # head/copy/blend in halves so half 0's blend overlaps
                # half 1's head matmuls. Multi-tile programs pipeline
                # across tiles anyway, so keep one instruction per step.
                pm9c = ps.tile([128, ppb, 6], F32, name="pm9c", tag="ps")
                pmC = small.tile([128, ppb, 6], F32, name="pmC", tag="pmC")
                cbg = small.tile([128, ppb, 3], F32, name="cbg", tag="cbg")
                cfg = small.tile([128, ppb, 3], F32, name="cfg", tag="cfg")
                # Pool absorbs the blend in steady state; for a single
                # tile its q7 launch latency sits on the exit path, so
                # keep the tail on DVE there.
                beng = nc.gpsimd if (BLEND_GP and ntiles > 1) else nc.vector
                nhalf = 2 if ntiles == 1 and ppb >= 2 else 1
                hpb = ppb // nhalf
                for b in range(nhalf):
                    jsl = slice(b * hpb, (b + 1) * hpb)
                    for j in range(b * hpb, (b + 1) * hpb):
                        gsl = slice(j * 128, (j + 1) * 128)
                        nc.tensor.matmul(pm9c[:, j, 0:6], c2r[:, gsl], WC3,
                                         start=True, stop=True)
                    (nc.scalar.copy(out=pmC[:, jsl], in_=pm9c[:, jsl])
                     if PM_ENG == 'A'
                     else nc.vector.tensor_copy(out=pmC[:, jsl],
                                                in_=pm9c[:, jsl]))
                    beng.tensor_mul(
                        cbg[:, jsl], pmC[:, jsl, 0:3],
                        pmS[:, jsl, 0].unsqueeze(2).broadcast_to(
                            (128, hpb, 3)))
                    beng.tensor_mul(
                        cfg[:, jsl], pmC[:, jsl, 3:6],
                        pmS[:, jsl, 2].unsqueeze(2).broadcast_to(
                            (128, hpb, 3)))
                    beng.tensor_add(cbg[:, jsl], cbg[:, jsl], cfg[:, jsl])
                    beng.tensor_mul(
                        out_sb[:, jsl, 0:3], cbg[:, jsl],
                        inv[:, jsl].unsqueeze(2).broadcast_to((128, hpb, 3)))

                o_dram = out[t * 128:(t + 1) * 128, :].rearrange(
                    "p (j c) -> p j c", c=6)
                nc.sync.dma_start(out=o_dram, in_=out_sb)

    if split_multiwait:
        _split_multiwait_instructions(nc)
    return nc


def kernel(**inputs):
    global LAST_RESULT
    x = np.asarray(inputs['x'], dtype=np.float32)
    n_total = x.shape[0]
    per_core = (n_total + N_CORES - 1) // N_CORES
    tile_pts = min(TILE_PTS, max(128, per_core))
    ntiles = (per_core + tile_pts - 1) // tile_pts
    padded = ntiles * tile_pts

    key = padded
    if key not in _PROG_CACHE:
        _PROG_CACHE[key] = _build_program(padded)
    nc = _PROG_CACHE[key]

    w = _prep_weights({k: v for k, v in inputs.items() if k != 'x'})

    in_maps = []
    for c in range(N_CORES):
        lo = c * per_core
        hi = min(lo + per_core, n_total)
        xc = np.zeros((NF, padded), np.float16)
        xc[:, :hi - lo] = x[lo:hi].T
        in_maps.append({'xin': xc, **w})

    trace = bool(int(os.environ.get('NERF_TRACE', '0')))
    res = run_bass_kernel_spmd(nc, in_maps, list(range(N_CORES)), trace=trace)
    LAST_RESULT = res

    ppb = tile_pts // 128
    pieces = []
    for c in range(N_CORES):
        lo = c * per_core
        hi = min(lo + per_core, n_total)
        o = res.results[c]['out'].reshape(ntiles, 128, ppb, 6)
        o = o.transpose(0, 2, 1, 3).reshape(padded, 6)
        pieces.append(o[:hi - lo])
    return np.concatenate(pieces, axis=0)
